# revision 2
# baseline (speedup 1.0000x reference)
"""LiquidRNN (LTC) kernel for 8 Trainium2 NeuronCores.

Strategy (data-parallel over batch, per the sharding hint):
  - B=32 batch rows -> 4 per core; weights replicated; time scan sequential per core.
  - Host folds the input mapping into the cell: W_comb = W_x @ W_in,
    b_comb = W_x @ b_in + b_cell, so the device computes
        G = x @ W_comb.T + b_comb                       (big parallel matmul)
        h = (h + ts*f*A) / (1 + ts*(inv_tau + f)),  f = sigmoid(G_t + h @ W_h.T)
        out = h_motor @ W_out.T + b_out                 (big parallel matmul)
  - All state kept transposed in SBUF: h^T[p, (m, b)] where m = hidden chunk of 128,
    so per-step work is [128, 32] tiles (full partition occupancy).
  - Matmul operands bf16 (fp32 PSUM accumulation); h master state fp32.

Self-contained: hardcodes shapes from the problem spec.
"""

import numpy as np

import concourse.bass as bass
from concourse.alu_op_type import AluOpType as ALU
import concourse.bacc as bacc
import concourse.mybir as mybir
import concourse.tile as tile
from concourse.bass_utils import run_bass_kernel_spmd

F32 = mybir.dt.float32
BF16 = mybir.dt.bfloat16
AF = mybir.ActivationFunctionType

NCORES = 8
B = 32
T = 1024
IN = 512
H = 1024
MOTOR = 512
OUT = 512

B_LOC = B // NCORES    # 4
NK = H // 128          # 8
NM = H // 128          # 8
NI = IN // 128         # 4
NMO = MOTOR // 128     # 4
HC = B_LOC * NM        # 32


def _build(t_win=128):
    nc = bacc.Bacc("TRN2", target_bir_lowering=False, debug=False, num_devices=1)

    x_d = nc.dram_tensor("x", [NI, 128, B_LOC, T], BF16, kind="ExternalInput")
    tsb_d = nc.dram_tensor("tsb", [T * HC], F32, kind="ExternalInput")
    h0_d = nc.dram_tensor("h0", [128, NM, B_LOC], F32, kind="ExternalInput")
    WcT_d = nc.dram_tensor("WcT", [IN, H], BF16, kind="ExternalInput")
    WhT_d = nc.dram_tensor("WhT", [H, H], BF16, kind="ExternalInput")
    WoT_d = nc.dram_tensor("WoT", [MOTOR, OUT], BF16, kind="ExternalInput")
    bc_d = nc.dram_tensor("bc", [128, NM], F32, kind="ExternalInput")
    Ab_d = nc.dram_tensor("Ab", [128, NM], F32, kind="ExternalInput")
    itb_d = nc.dram_tensor("itb", [128, NM], F32, kind="ExternalInput")
    bo_d = nc.dram_tensor("bo", [1, OUT], F32, kind="ExternalInput")
    out_d = nc.dram_tensor("out", [T * B_LOC, OUT], F32, kind="ExternalOutput")
    last_d = nc.dram_tensor("last", [128, HC], F32, kind="ExternalOutput")

    from contextlib import ExitStack

    with tile.TileContext(nc) as tc, ExitStack() as ctx:
        const = ctx.enter_context(tc.tile_pool(name="const", bufs=1))
        big = ctx.enter_context(tc.tile_pool(name="big", bufs=1))

        Wh_sb = const.tile([128, NK * NM * 128], BF16)
        for k in range(NK):
            nc.sync.dma_start(
                Wh_sb[:, k * NM * 128 : (k + 1) * NM * 128],
                WhT_d.ap()[k * 128 : (k + 1) * 128, :],
            )
        Wc_sb = const.tile([128, NI * NM * 128], BF16)
        for i in range(NI):
            nc.sync.dma_start(
                Wc_sb[:, i * NM * 128 : (i + 1) * NM * 128],
                WcT_d.ap()[i * 128 : (i + 1) * 128, :],
            )
        Wo_sb = const.tile([128, NMO * OUT], BF16)
        for m in range(NMO):
            nc.sync.dma_start(
                Wo_sb[:, m * OUT : (m + 1) * OUT],
                WoT_d.ap()[m * 128 : (m + 1) * 128, :],
            )
        bc_sb = const.tile([128, NM], F32)
        nc.sync.dma_start(bc_sb[:], bc_d.ap())
        Ab_sb = const.tile([128, NM], F32)
        nc.sync.dma_start(Ab_sb[:], Ab_d.ap())
        itb_sb = const.tile([128, NM], F32)
        nc.sync.dma_start(itb_sb[:], itb_d.ap())
        bo_sb = const.tile([1, OUT], F32)
        nc.sync.dma_start(bo_sb[:], bo_d.ap())
        ones_sb = const.tile([1, 128], BF16)
        nc.vector.memset(ones_sb[:], 1.0)
        bo_bf = const.tile([1, OUT], BF16)
        nc.vector.tensor_copy(bo_bf[:], bo_sb[:])

        G_sb = big.tile([128, T * HC], BF16)
        H_sb = big.tile([128, T * B_LOC * NMO], BF16)

        # ---------------- G phase ----------------
        with (
            tc.tile_pool(name="gx", bufs=3) as gx,
            tc.tile_pool(name="gps", bufs=8, space="PSUM") as gps,
        ):
            NW = T // t_win
            ncols = t_win * B_LOC
            G4 = G_sb[:].rearrange("p (t m b) -> p t m b", m=NM, b=B_LOC)
            for w in range(NW):
                xt = [gx.tile([128, ncols], BF16, tag=f"x{i}", name=f"xt{i}") for i in range(NI)]
                for i in range(NI):
                    nc.sync.dma_start(
                        xt[i][:].rearrange("p (b t) -> p b t", b=B_LOC),
                        x_d.ap()[i, :, :, w * t_win : (w + 1) * t_win],
                    )
                for m in range(NM):
                    ps = gps.tile([128, ncols], F32, tag="gp")
                    for i in range(NI):
                        nc.tensor.matmul(
                            ps[:],
                            Wc_sb[:, (i * NM + m) * 128 : (i * NM + m + 1) * 128],
                            xt[i][:],
                            start=(i == 0),
                            stop=(i == NI - 1),
                        )
                    dst = G4[:, w * t_win : (w + 1) * t_win, m, :]
                    nc.vector.tensor_scalar(
                        dst, ps[:].rearrange("p (b t) -> p t b", b=B_LOC),
                        bc_sb[:, m : m + 1], None, ALU.add,
                    )

        # ---------------- scan ----------------
        with (
            tc.tile_pool(name="hst", bufs=2) as hst,
            tc.tile_pool(name="tmp", bufs=3) as tmp,
            tc.tile_pool(name="tsw", bufs=2) as tswp,
            tc.tile_pool(name="zps", bufs=2, space="PSUM") as zps,
        ):
            H5 = H_sb[:].rearrange("p (m t b) -> p m t b", m=NMO, t=T)
            h32 = hst.tile([128, HC], F32, tag="h32")
            nc.sync.dma_start(h32[:], h0_d.ap().rearrange("p m b -> p (m b)"))
            hbf = hst.tile([128, HC], BF16, tag="hbf")
            nc.vector.tensor_copy(hbf[:], h32[:])

            ts_win_steps = min(128, T)
            itb_bc = itb_sb[:].unsqueeze(2).broadcast_to([128, NM, B_LOC])
            Ab_bc = Ab_sb[:].unsqueeze(2).broadcast_to([128, NM, B_LOC])

            for t in range(T):
                tw, tloc = divmod(t, ts_win_steps)
                if tloc == 0:
                    tsw = tswp.tile([128, ts_win_steps * HC], F32, tag="ts")
                    src = tsb_d.ap()[tw * ts_win_steps * HC : (tw + 1) * ts_win_steps * HC]
                    nc.sync.dma_start(tsw[:], src.partition_broadcast(128))
                ts_t = tsw[:, tloc * HC : (tloc + 1) * HC]

                ps = zps.tile([128, HC], F32, tag="z")
                for m in range(NM):
                    po = ps[:, m * B_LOC : (m + 1) * B_LOC]
                    for k in range(NK):
                        nc.tensor.matmul(
                            po,
                            Wh_sb[:, (k * NM + m) * 128 : (k * NM + m + 1) * 128],
                            hbf[:, k * B_LOC : (k + 1) * B_LOC],
                            start=(k == 0),
                            stop=(k == NK - 1),
                        )
                zf = tmp.tile([128, HC], F32, tag="zf")
                nc.vector.tensor_add(zf[:], ps[:], G_sb[:, t * HC : (t + 1) * HC])
                f = tmp.tile([128, HC], F32, tag="f")
                nc.scalar.activation(f[:], zf[:], AF.Sigmoid)
                s = tmp.tile([128, HC], F32, tag="s")
                nc.vector.tensor_add(
                    s[:].rearrange("p (m b) -> p m b", b=B_LOC),
                    f[:].rearrange("p (m b) -> p m b", b=B_LOC),
                    itb_bc,
                )
                p2 = tmp.tile([128, HC], F32, tag="p2")
                nc.vector.tensor_mul(p2[:], s[:], ts_t)
                den = tmp.tile([128, HC], F32, tag="den")
                nc.scalar.activation(den[:], p2[:], AF.Identity, bias=1.0)
                r = tmp.tile([128, HC], F32, tag="r")
                nc.vector.reciprocal_approx_fast(r[:], den[:])
                t1 = tmp.tile([128, HC], F32, tag="t1")
                nc.vector.tensor_mul(t1[:], f[:], ts_t)
                n1 = tmp.tile([128, HC], F32, tag="n1")
                nc.vector.tensor_mul(
                    n1[:].rearrange("p (m b) -> p m b", b=B_LOC),
                    t1[:].rearrange("p (m b) -> p m b", b=B_LOC),
                    Ab_bc,
                )
                num = tmp.tile([128, HC], F32, tag="num")
                nc.vector.tensor_add(num[:], n1[:], h32[:])
                h32 = hst.tile([128, HC], F32, tag="h32")
                nc.vector.tensor_mul(h32[:], num[:], r[:])
                hbf = hst.tile([128, HC], BF16, tag="hbf")
                nc.vector.tensor_mul(hbf[:], num[:], r[:])
                nc.scalar.copy(
                    H5[:, :, t, :],
                    hbf[:, : B_LOC * NMO].rearrange("p (m b) -> p m b", b=B_LOC),
                )

            nc.sync.dma_start(last_d.ap(), h32[:])

        # ---------------- out phase ----------------
        with (
            tc.tile_pool(name="osb", bufs=3) as osb,
            tc.tile_pool(name="ops", bufs=2, space="PSUM") as ops,
        ):
            TC = 128 // B_LOC
            for c in range(T // TC):
                ps = ops.tile([128, OUT], F32, tag="op")
                nc.tensor.matmul(ps[:], ones_sb[:], bo_bf[:], start=True, stop=False)
                for m in range(NMO):
                    lhsT = H_sb[:, m * T * B_LOC + c * 128 : m * T * B_LOC + (c + 1) * 128]
                    nc.tensor.matmul(
                        ps[:], lhsT, Wo_sb[:, m * OUT : (m + 1) * OUT],
                        start=False, stop=(m == NMO - 1),
                    )
                ot = osb.tile([128, OUT], F32, tag="ot")
                nc.vector.tensor_copy(ot[:], ps[:])
                nc.sync.dma_start(out_d.ap()[c * 128 : (c + 1) * 128, :], ot[:])

    nc.compile()
    return nc


def _host_prep(inputs):
    import ml_dtypes

    bf = ml_dtypes.bfloat16
    x = np.asarray(inputs["inputs"], dtype=np.float32)
    ts = np.asarray(inputs["timespans"], dtype=np.float32)
    h0 = np.asarray(inputs["init_hidden"], dtype=np.float32)
    W_x = np.asarray(inputs["W_x"], dtype=np.float32)
    W_in = np.asarray(inputs["W_in"], dtype=np.float32)
    W_comb = (W_x @ W_in).astype(np.float32)
    b_comb = (W_x @ np.asarray(inputs["b_in"], dtype=np.float32)
              + np.asarray(inputs["b_cell"], dtype=np.float32)).astype(np.float32)
    inv_tau = (1.0 / np.asarray(inputs["tau"], dtype=np.float32)).astype(np.float32)
    A = np.asarray(inputs["A"], dtype=np.float32)

    WcT = np.ascontiguousarray(W_comb.T).astype(bf)
    WhT = np.ascontiguousarray(np.asarray(inputs["W_h"], dtype=np.float32).T).astype(bf)
    WoT = np.ascontiguousarray(np.asarray(inputs["W_out"], dtype=np.float32).T).astype(bf)
    bc = np.ascontiguousarray(b_comb.reshape(NM, 128).T)
    Ab = np.ascontiguousarray(A.reshape(NM, 128).T)
    itb = np.ascontiguousarray(inv_tau.reshape(NM, 128).T)
    bo = np.asarray(inputs["b_out"], dtype=np.float32).reshape(1, OUT)

    in_maps = []
    for c in range(NCORES):
        sl = slice(c * B_LOC, (c + 1) * B_LOC)
        xc = x[sl]
        xT = np.ascontiguousarray(xc.transpose(2, 0, 1).reshape(NI, 128, B_LOC, T))
        tsl = ts[sl]  # [B_LOC, T]
        tsb = np.ascontiguousarray(
            np.broadcast_to(tsl.T[:, None, :], (T, NM, B_LOC))
        ).reshape(-1).astype(np.float32)
        h0T = np.ascontiguousarray(
            h0[sl].T.reshape(NM, 128, B_LOC).transpose(1, 0, 2)
        ).astype(np.float32)
        in_maps.append({
            "x": xT.astype(bf), "tsb": tsb, "h0": h0T,
            "WcT": WcT, "WhT": WhT, "WoT": WoT,
            "bc": bc, "Ab": Ab, "itb": itb, "bo": bo,
        })
    return in_maps


_NC_CACHE = {}


def _get_nc():
    if "nc" not in _NC_CACHE:
        _NC_CACHE["nc"] = _build()
    return _NC_CACHE["nc"]


def kernel(**inputs):
    nc = _get_nc()
    in_maps = _host_prep(inputs)
    res = run_bass_kernel_spmd(nc, in_maps, core_ids=list(range(NCORES)))

    out = np.empty((B, T, OUT), np.float32)
    last = np.empty((B, H), np.float32)
    for c in range(NCORES):
        r = res.results[c]
        out[c * B_LOC : (c + 1) * B_LOC] = (
            r["out"].reshape(T, B_LOC, OUT).transpose(1, 0, 2)
        )
        last[c * B_LOC : (c + 1) * B_LOC] = (
            r["last"].reshape(128, NM, B_LOC).transpose(2, 1, 0).reshape(B_LOC, H)
        )
    return out, last
